# revision 7
# baseline (speedup 1.0000x reference)
"""AudioSNN Trainium2 kernel.

Two-layer leaky-integrate-and-fire SNN (snntorch Leaky, reset-by-subtract),
T=500 recurrent steps over batch 4096, data-parallel over 8 NeuronCores
(512 batch elements per core).

Math (per step t, reference):
    cur1 = x_t @ W1.T + b1
    m1   = beta*m1 + cur1 - H(m1_prev - 1)        # H(m1_prev-1) == spk1[t-1]
    spk1 = H(m1 - 1)
    cur2 = spk1 @ W2.T + b2
    m2   = beta*m2 + cur2 - spk2[t-1]
    spk2 = H(m2 - 1)    -> output [T, B, 5]

Device formulation (per core, batch 512 split in 2 halves of 256):
  L1 state z1 = m1 - 1, spikes in sign form sgn = sign(z1) (spk = (sgn+1)/2):
    psum1 = W1aug^T.T @ x_aug   (fp32 matmul; bias b1+beta-1.5 via ones-row)
    psum1 += (-0.5*I) @ sgn[t-1]  (bf16 matmul "inject" = -spk + 1/2-const)
    z1[t] = beta*z1[t-1] + psum1  (one DVE scalar_tensor_tensor)
    sgn[t] = Sign(z1[t]) -> bf16  (ACT engine)
  L2 runs transposed ([batch 128, 4 quarters x 5 outputs] on free dim),
  state z2~ = m2 - 1 - p with p = C2/(1-beta), C2 = 0.5*sum(W2,h) + b2 + beta-1:
    psum2[:, q*5:(q+1)*5] = sgn_q^T @ (0.5*W2^T)   (bf16, hi+lo split, FWL)
    psum2 += (-I) @ spk2[t-1]
    z2[t] = beta*z2[t-1] + psum2  (DVE)
    spk2[t] = (z2[t] > -p)        (DVE tensor_tensor is_gt -> bf16, = output)
"""

import os
import sys

sys.path.insert(0, "/opt/trn_rl_repo")

from contextlib import ExitStack

import numpy as np
import ml_dtypes

from concourse import bacc, mybir, tile
from concourse.bass_utils import run_bass_kernel_spmd

BETA = 0.9
T, F, H, O = 500, 40, 128, 5
NCORES = 8
BC = 512  # batch per core
HB = 256  # half-batch
CH = 20  # time steps per DMA chunk (must divide T)
F32 = mybir.dt.float32
BF16 = mybir.dt.bfloat16
BF16_NP = ml_dtypes.bfloat16

MULT = mybir.AluOpType.mult
ADD = mybir.AluOpType.add
IS_GT = mybir.AluOpType.is_gt


def build(nc, n_steps=T, ch=CH):
    """Emit the per-core program. x_aug layout: [n_chunks, (F+1)*ch*BC]."""
    n_chunks = n_steps // ch
    OD = 4 * O  # 20: free width of the transposed layer-2 tiles

    x_d = nc.dram_tensor(
        "x_aug", [n_chunks, (F + 1) * ch * BC], F32, kind="ExternalInput"
    ).ap()
    w1_d = nc.dram_tensor("w1aug", [F + 1, H], F32, kind="ExternalInput").ap()
    nhi_d = nc.dram_tensor("neg_half_i", [H, H], BF16, kind="ExternalInput").ap()
    ni_d = nc.dram_tensor("neg_i", [H, H], BF16, kind="ExternalInput").ap()
    w2h_d = nc.dram_tensor("w2hi", [H, O], BF16, kind="ExternalInput").ap()
    w2l_d = nc.dram_tensor("w2lo", [H, O], BF16, kind="ExternalInput").ap()
    npp_d = nc.dram_tensor("negp", [H, OD], F32, kind="ExternalInput").ap()
    z2i_d = nc.dram_tensor("z2init", [H, OD], F32, kind="ExternalInput").ap()
    out_d = nc.dram_tensor("out", [H, n_steps * OD], BF16, kind="ExternalOutput").ap()

    with tile.TileContext(nc) as tc, ExitStack() as ctx:
        const = ctx.enter_context(tc.tile_pool(name="const", bufs=1))
        state = ctx.enter_context(tc.tile_pool(name="state", bufs=1))
        xin = ctx.enter_context(tc.tile_pool(name="xin", bufs=3))
        outp = ctx.enter_context(tc.tile_pool(name="outp", bufs=3))
        ps1 = ctx.enter_context(tc.tile_pool(name="ps1", bufs=2, space="PSUM"))
        ps2 = ctx.enter_context(tc.tile_pool(name="ps2", bufs=2, space="PSUM"))

        w1_s = const.tile([F + 1, H], F32, tag="w1")
        nhi_s = const.tile([H, H], BF16, tag="nhi")
        ni_s = const.tile([H, H], BF16, tag="ni")
        w2h_s = const.tile([H, O], BF16, tag="w2h")
        w2l_s = const.tile([H, O], BF16, tag="w2l")
        npp_s = const.tile([H, OD], F32, tag="npp")
        for s, d in [
            (w1_s, w1_d),
            (nhi_s, nhi_d),
            (ni_s, ni_d),
            (w2h_s, w2h_d),
            (w2l_s, w2l_d),
            (npp_s, npp_d),
        ]:
            nc.sync.dma_start(out=s[:], in_=d[:])

        # Recurrent state, ping-pong buffered (index = t % 2).
        z1 = [
            [state.tile([H, HB], F32, tag=f"z1_{h}_{pp}", name=f"z1_{h}_{pp}") for pp in range(2)]
            for h in range(2)
        ]
        sg = [
            [state.tile([H, HB], BF16, tag=f"sg_{h}_{pp}", name=f"sg_{h}_{pp}") for pp in range(2)]
            for h in range(2)
        ]
        z2 = [state.tile([H, OD], F32, tag=f"z2_{pp}", name=f"z2_{pp}") for pp in range(2)]
        spk0 = state.tile([H, OD], BF16, tag="spk0")

        for h in range(2):
            nc.vector.memset(z1[h][1][:], -1.0)  # m1(0)=0 -> z1=-1
            nc.vector.memset(sg[h][1][:], -1.0)  # sign(-1)
        nc.sync.dma_start(out=z2[1][:], in_=z2i_d[:])
        nc.vector.memset(spk0[:], 0.0)

        xt = None
        ot = None
        spk_prev = spk0[:]
        for t in range(n_steps):
            chk, st = divmod(t, ch)
            if st == 0:
                xt = xin.tile([F + 1, ch * BC], F32, tag="xt")
                nc.sync.dma_start(out=xt[:], in_=x_d[chk : chk + 1, :])
                ot = outp.tile([H, ch * OD], BF16, tag="ot")
            cur, prv = t % 2, 1 - (t % 2)

            # ---- layer 1 (two independent batch halves) ----
            for h in range(2):
                p1 = ps1.tile([H, HB], F32, tag=f"p1_{h}")
                xs = xt[:, st * BC + h * HB : st * BC + (h + 1) * HB]
                nc.tensor.matmul(p1[:], w1_s[:], xs, start=True, stop=False)
                nc.tensor.matmul(
                    p1[:], nhi_s[:], sg[h][prv][:], start=False, stop=True
                )
                nc.vector.scalar_tensor_tensor(
                    z1[h][cur][:], z1[h][prv][:], BETA, p1[:], MULT, ADD
                )
                nc.scalar.sign(sg[h][cur][:], z1[h][cur][:])

            # ---- layer 2 (transposed: psum2[b, q*5+o]) ----
            p2 = ps2.tile([H, OD], F32, tag="p2")
            nc.tensor.matmul(p2[:], ni_s[:], spk_prev, start=True, stop=False)
            for q in range(4):
                h, qq = divmod(q, 2)
                sgq = sg[h][cur][:, qq * H : (qq + 1) * H]
                r = p2[:, q * O : (q + 1) * O]
                nc.tensor.matmul(r, sgq, w2h_s[:], start=False, stop=False)
                nc.tensor.matmul(r, sgq, w2l_s[:], start=False, stop=(q == 3))
            nc.vector.scalar_tensor_tensor(
                z2[cur][:], z2[prv][:], BETA, p2[:], MULT, ADD
            )
            o_slice = ot[:, st * OD : (st + 1) * OD]
            nc.vector.tensor_tensor(o_slice, z2[cur][:], npp_s[:], IS_GT)
            spk_prev = o_slice

            if st == ch - 1:
                nc.sync.dma_start(
                    out=out_d[:, chk * ch * OD : (chk + 1) * ch * OD], in_=ot[:]
                )


def host_inputs(x, W1, b1, W2, b2, n_steps=T, ch=CH):
    """Shard + precompute all per-core device input arrays."""
    n_chunks = n_steps // ch
    x = np.asarray(x, np.float32)[:, :n_steps, :]
    W1 = np.asarray(W1, np.float32)
    b1 = np.asarray(b1, np.float32)
    W2 = np.asarray(W2, np.float32)
    b2 = np.asarray(b2, np.float32)

    # x: [B, T', F] -> per core [T', F, 512] -> augment ones -> chunked
    xs = x.reshape(NCORES, BC, n_steps, F).transpose(0, 2, 3, 1)  # [8,T',40,512]
    aug = np.empty((NCORES, n_steps, F + 1, BC), np.float32)
    aug[:, :, :F, :] = xs
    aug[:, :, F, :] = 1.0
    # [8, T', 41, 512] -> [8, n_chunks, 41, ch, 512] (chunk-major, partition dim 41)
    aug = aug.reshape(NCORES, n_chunks, ch, F + 1, BC).transpose(0, 1, 3, 2, 4)
    aug = np.ascontiguousarray(aug).reshape(NCORES, n_chunks, (F + 1) * ch * BC)

    w1aug = np.concatenate([W1.T, (b1 + BETA - 1.5)[None, :]], axis=0)  # [41,128]

    eye = np.eye(H, dtype=np.float32)
    neg_half_i = (-0.5 * eye).astype(BF16_NP)
    neg_i = (-eye).astype(BF16_NP)

    w2half = (0.5 * W2.T).astype(np.float32)  # [128, 5]
    w2hi = w2half.astype(BF16_NP)
    w2lo = (w2half - w2hi.astype(np.float32)).astype(BF16_NP)

    s2 = 0.5 * W2.sum(axis=1)  # [5]
    C2 = s2 + b2 + BETA - 1.0
    p = C2 / (1.0 - BETA)
    negp = np.tile(-p, 4)[None, :].repeat(H, 0).astype(np.float32)  # [128,20]
    z2init = np.tile(-1.0 - p, 4)[None, :].repeat(H, 0).astype(np.float32)

    shared = {
        "w1aug": np.ascontiguousarray(w1aug),
        "neg_half_i": neg_half_i,
        "neg_i": neg_i,
        "w2hi": w2hi,
        "w2lo": w2lo,
        "negp": negp,
        "z2init": z2init,
    }
    return [{"x_aug": aug[c], **shared} for c in range(NCORES)]


def assemble(results, n_steps=T):
    """[H, T'*20] bf16 per core -> [T', B, O] float32."""
    OD = 4 * O
    outs = []
    for r in results:
        a = np.asarray(r["out"]).reshape(H, n_steps, 4, O).astype(np.float32)
        outs.append(a.transpose(1, 2, 0, 3).reshape(n_steps, 4 * H, O))
    return np.concatenate(outs, axis=1)


LAST_RESULT = None  # BassKernelResults of the most recent run (for profiling)


def kernel(x, W1, b1, W2, b2):
    global LAST_RESULT
    in_maps = host_inputs(x, W1, b1, W2, b2)
    nc = bacc.Bacc("TRN2", target_bir_lowering=False, debug=False)
    build(nc)
    nc.compile()
    LAST_RESULT = run_bass_kernel_spmd(nc, in_maps, list(range(NCORES)))
    return assemble(LAST_RESULT.results)
